# revision 40
# baseline (speedup 1.0000x reference)
"""Multi-head attention (B=2, S=2048, DIM=512, H=8) on 8 Trainium2 cores.

Sharding: data-parallel over batch x tensor-parallel over heads.
Core c handles batch b = c // 4 and heads {2g, 2g+1} where g = c % 4
(i.e. output feature columns [128g : 128g+128]).  All sharding /
gathering happens host-side; no on-device collectives.

Per-core kernel.  ScalarE's exp train (64 [128,1024] Exp activations,
~1.0-1.2us issue each depending on the host's clock state) and the PE
(scores + ctx matmuls, ~1.0-1.1us per key tile) are co-critical, so
the emission order weaves exactly one score-tile unit per exp slot
with ~2 ctx-tile units of filler, and every filler lands where its
inputs (DMA arrivals, es tiles, PSUM hand-offs) are already resolved:
  - ctx consumption lags its exp by ~8 slots so a ctx matmul never
    waits on a just-finishing exp (measured ~370ns/slot when lag=2),
  - score batches never exceed the 2-deep score-PSUM ring, so a
    score matmul's WAR wait never blocks ctx work behind it.

Key structural choices:
  - q/k projection path in fp8-e3m4 (inputs AND weights; weights
    host-scaled by 16 into fp8's normal range, the 1/16^2 folded into
    the exp's fused scale); the v path stays fp16,
  - inputs host-prearranged so each 512-seq chunk is one
    partition-contiguous 2KB-per-line transfer, split across the two
    HWDGE queues in need-order (consts first; the K side rides the
    shorter scalar queue so the whole K chunk-0 projection runs
    during the xq0 wait; the scalar queue's issue instructions all
    retire before the first exp),
  - PE and DVE warmed on memset scratch tiles through the chunk-0
    DMA wait (cold-clock projections measured ~2x slower); the exp
    activation table preloaded with a dummy exp,
  - output kept in the attention-native transposed layout end to end:
    the ones-row denominator of ctx^T PSUM [65,512] is broadcast
    across partitions 0:64 by a 1-row fp16 matmul, reciprocated in
    one custom-DVE pass (reciprocal_approx_fast, which only works at
    base partition 0 on this hardware; the builtin reciprocal is ~8
    cycles/elem), multiplied against the ctx PSUM, and stored with
    one [64,512] DMA per (q-block, head); the host untransposes.

Compute structure:
  - Q^T, K^T projections in [out_dim(128), seq] layout (head h at
    partitions 64h..64h+63) - attention-ready; V in natural
    [seq, out_dim] tiles with a ones column per head so the ctx
    matmul also accumulates the softmax denominator for free,
  - scores^T = K_h @ Q_h^T per 128-row key tile (K=64), the two
    heads' matmuls target disjoint PE row groups and run
    concurrently; exp on ScalarE with the 1/sqrt(512) scale fused
    ([128,1024] tiles, fp16 output),
  - ctx^T accumulated over key tiles (lhsT = V tile [128,65],
    rhs = exp-scores [128,512], fp32 PSUM).
"""

import os

import ml_dtypes
import numpy as np

DIM = 512
NUM_HEADS = 8
D_HEAD = 64
B = 2
S = 2048
N_CORES = 8
P = 128  # partitions
NK = DIM // P  # 4 contraction tiles for projections
NT = S // P  # 16 key tiles
VSTRIDE = 132  # V tile stride: [h0(64) | ones | h1(64) | ones | 2 pad]
CH = 512  # input DMA / projection chunk (columns of seq)
NC_ = S // CH  # 4 chunks
# q/k path runs in fp8-e3m4: host scales Wq/Wk (and bq/bk) by WSCALE so
# the weights sit in fp8's normal range; scores come out scaled by
# WSCALE^2, folded into the exp's fused scale.
WSCALE = 16.0
SCALE = float(1.0 / np.sqrt(512.0) / (WSCALE * WSCALE))

_CACHE = {}


def _build_program():
    import concourse.tile as tile
    from concourse import bacc, mybir

    f32 = mybir.dt.float32
    f16 = mybir.dt.float16
    f8 = mybir.dt.float8e3
    nc = bacc.Bacc("TRN2", target_bir_lowering=False, debug=False)

    io = {}
    # [p, c*2048 + k*512 + s] = x[c*512+s, 128k+p]
    io["xq"] = nc.dram_tensor("xq", [P, NK * S], f8, kind="ExternalInput").ap()
    io["xk"] = nc.dram_tensor("xk", [P, NK * S], f8, kind="ExternalInput").ap()
    io["xv"] = nc.dram_tensor("xv", [P, NK * S], f16, kind="ExternalInput").ap()
    io["wq"] = nc.dram_tensor("wq", [P, DIM], f8, kind="ExternalInput").ap()
    io["wk"] = nc.dram_tensor("wk", [P, DIM], f8, kind="ExternalInput").ap()
    io["wv"] = nc.dram_tensor("wv", [P, DIM], f16, kind="ExternalInput").ap()
    io["bqk"] = nc.dram_tensor("bqk", [P, 2], f32, kind="ExternalInput").ap()
    io["bvb"] = nc.dram_tensor("bvb", [P, P], f32, kind="ExternalInput").ap()
    # [d', q] = out_natural[q, d']  (d' = 64h + d within this core's 128 cols)
    io["out"] = nc.dram_tensor("out", [P, S], f32, kind="ExternalOutput").ap()

    with tile.TileContext(nc) as tc:
        _emit(tc, mybir, io)
    nc.compile()
    return nc


def _emit(tc, mybir, io):
    from contextlib import ExitStack

    nc = tc.nc
    f32 = mybir.dt.float32
    f16 = mybir.dt.float16
    f8 = mybir.dt.float8e3
    Exp = mybir.ActivationFunctionType.Exp

    mm = nc.tensor.matmul

    with ExitStack() as ctx:
        const = ctx.enter_context(tc.tile_pool(name="const", bufs=1))
        qk = ctx.enter_context(tc.tile_pool(name="qk", bufs=1))
        vpool = ctx.enter_context(tc.tile_pool(name="vpool", bufs=1))
        opool = ctx.enter_context(tc.tile_pool(name="opool", bufs=2))
        rpool = ctx.enter_context(tc.tile_pool(name="rpool", bufs=2))
        csbpool = ctx.enter_context(tc.tile_pool(name="csbp", bufs=2))

        # constants ride the fast HWDGE queues ahead of the input
        # stream (the SWDGE proved an order of magnitude slower); only
        # bvb (needed latest) stays on gpsimd
        wq_sb = const.tile([P, DIM], f8, tag="wq")
        wk_sb = const.tile([P, DIM], f8, tag="wk")
        wv_sb = const.tile([P, DIM], f16, tag="wv")
        bqk_sb = const.tile([P, 2], f32, tag="bqk")
        bq_sb = bqk_sb[:, 0:1]
        bk_sb = bqk_sb[:, 1:2]
        bvb_sb = const.tile([P, P], f32, tag="bvb")
        scratch = const.tile([P, CH], f16, tag="scratch")
        ones_sb = const.tile([65, 64], f16, tag="ones")
        dume = const.tile([1, 8], f16, tag="dume")
        nc.gpsimd.dma_start(bvb_sb[:], io["bvb"][:])

        # warmup fodder + the broadcast ones row + exp-table preload input
        nc.vector.memset(scratch[:], 0.5)
        nc.vector.memset(ones_sb[:], 1.0)
        # keep the DVE clock up through the DMA wait so the first
        # QT/KT bias-adds run warm (measured 898ns cold vs ~450 warm);
        # separate tile so the PE warmup stream is not serialized
        # against these rewrites
        scratch2 = const.tile([P, CH], f16, tag="scratch2")
        for _ in range(12):
            nc.vector.memset(scratch2[:], 0.5)

        # persistent projection outputs
        QT = qk.tile([P, S], f16, tag="QT")  # [out_dim, seq]
        KT = qk.tile([P, S], f16, tag="KT")
        V = vpool.tile([P, NT * VSTRIDE], f16, tag="V")  # 16 x [128, 132]

        with (
            tc.tile_pool(name="xin", bufs=4) as xin,
            tc.tile_pool(name="psq", bufs=2, space="PSUM") as psq,
            tc.tile_pool(name="pss", bufs=2, space="PSUM") as pss,
            tc.tile_pool(name="psc", bufs=1, space="PSUM") as psc,
            tc.tile_pool(name="es", bufs=16) as espool,
        ):
            # ones columns of V (one per head per key tile, both at
            # local column 64 so the denominator is row 64 of ctx^T
            # for either head)
            nc.vector.memset(
                V[:].rearrange("p (t c) -> p t c", c=VSTRIDE)[:, :, 64:65], 1.0
            )
            nc.vector.memset(
                V[:].rearrange("p (t c) -> p t c", c=VSTRIDE)[:, :, 129:130], 1.0
            )
            # preload the Exp activation table while DMAs stream
            nc.scalar.activation(dume[:], scratch[0:1, 0:8], Exp, scale=SCALE)

            # ---- input DMA: all issued upfront, need-ordered, full-chunk
            # 2KB-per-line transfers striped across the two idle HWDGE
            # queues (sync + vector).  The scalar queue carries nothing.
            xt = {}  # (kind, c) -> tile
            for c in range(NC_):
                xt["q", c] = xin.tile([P, NK * CH], f8, tag=f"xq{c}", name="xt")
                xt["k", c] = xin.tile([P, NK * CH], f8, tag=f"xk{c}", name="xt")
                xt["v", c] = xin.tile([P, NK * CH], f16, tag=f"xv{c}", name="xt")

            def issue(queue, kind, c, lo=0, hi=NK * CH):
                key = {"q": "xq", "k": "xk", "v": "xv"}[kind]
                w = NK * CH
                queue.dma_start(
                    xt[kind, c][:, lo:hi], io[key][:, c * w + lo : c * w + hi]
                )

            H = NK * CH // 2
            # the two HWDGE queues drain in order and concurrently.
            # xq0 leads the sync queue with zero bytes ahead of it;
            # both weight tensors ride the scalar queue ahead of xk0
            # (they land before the Q projection needs them).  The
            # scalar queue's issue instructions all retire before the
            # first exp; everything else streams on sync in need-order.
            issue(nc.sync, "q", 0)
            nc.scalar.dma_start(wq_sb[:], io["wq"][:])
            nc.sync.dma_start(bqk_sb[:], io["bqk"][:])
            nc.scalar.dma_start(wk_sb[:], io["wk"][:])
            issue(nc.sync, "k", 1)
            issue(nc.scalar, "k", 0)
            issue(nc.sync, "v", 0, 0, H)
            issue(nc.scalar, "q", 1)
            issue(nc.sync, "v", 1)
            nc.scalar.dma_start(wv_sb[:], io["wv"][:])
            issue(nc.sync, "k", 2)
            issue(nc.scalar, "v", 0, H)
            issue(nc.sync, "k", 3)
            issue(nc.sync, "q", 2)
            issue(nc.sync, "q", 3)
            issue(nc.sync, "v", 2)
            issue(nc.sync, "v", 3)

            # ---- emission units -------------------------------------
            def warmup(n):
                # dummy matmuls on the memset scratch tile (present long
                # before the weights arrive): keeps the PE continuously
                # busy through the chunk-0 DMA wait so the HAM clock is
                # at full speed when the real projections start
                ps = psq.tile([P, CH], f32, tag="psq", name="warm")
                for i in range(n):
                    mm(ps[:], scratch[:, 0:P], scratch[:], start=True, stop=True)

            def proj_q(c):
                ps = psq.tile([P, CH], f32, tag="psq", name="psq")
                src = xt["q", c]
                for k in range(NK):
                    mm(
                        ps[:],
                        wq_sb[:, k * P : (k + 1) * P],
                        src[:, k * CH : (k + 1) * CH],
                        start=(k == 0),
                        stop=(k == NK - 1),
                    )
                nc.vector.tensor_scalar_add(
                    QT[:, c * CH : (c + 1) * CH], ps[:], bq_sb[:, 0:1]
                )

            def proj_k(c, lo=0, hi=CH):
                ps = psq.tile([P, CH], f32, tag="psq", name="psq")
                src = xt["k", c]
                for k in range(NK):
                    mm(
                        ps[:, 0 : hi - lo],
                        wk_sb[:, k * P : (k + 1) * P],
                        src[:, k * CH + lo : k * CH + hi],
                        start=(k == 0),
                        stop=(k == NK - 1),
                    )
                nc.vector.tensor_scalar_add(
                    KT[:, c * CH + lo : c * CH + hi],
                    ps[:, 0 : hi - lo],
                    bk_sb[:, 0:1],
                )

            def proj_v(c, i):
                # one V tile (natural layout + ones column)
                ti = c * (CH // P) + i
                ps = psq.tile([P, P], f32, tag="psq", name="psv")
                src = xt["v", c]
                for k in range(NK):
                    mm(
                        ps[:],
                        src[:, k * CH + i * P : k * CH + (i + 1) * P],
                        wv_sb[:, k * P : (k + 1) * P],
                        start=(k == 0),
                        stop=(k == NK - 1),
                    )
                o = ti * VSTRIDE
                # both heads in one strided add: dst views cols
                # [o..o+63] and [o+65..o+128] (skipping the shared
                # ones column) as a [2, 64] free pattern
                nc.vector.tensor_add(
                    V[:, o : o + 130].rearrange("p (a c) -> p a c", c=65)[
                        :, :, 0:64
                    ],
                    ps[:].rearrange("p (a c) -> p a c", c=64),
                    bvb_sb[:].rearrange("p (a c) -> p a c", c=64),
                )

            ess = {}  # (q, t) -> es tile

            def score(q, t):
                # one score tile pair (both heads, disjoint PE row
                # groups) + its exp
                qs = slice(q * 512, (q + 1) * 512)
                sps = pss.tile([P, 1024], f32, tag="sps", name="sps")
                for h in range(2):
                    hp = 64 * h
                    mm(
                        sps[:, h * 512 : (h + 1) * 512],
                        KT[hp : hp + 64, t * P : (t + 1) * P],
                        QT[hp : hp + 64, qs],
                        start=True,
                        stop=True,
                    )
                es = espool.tile([P, 1024], f16, tag="es", name="es")
                nc.scalar.activation(es[:], sps[:], Exp, scale=SCALE)
                ess[q, t] = es

            cps = {}

            def ctx(q, t):
                es = ess.pop((q, t))
                for h in range(2):
                    vo = t * VSTRIDE + 65 * h
                    mm(
                        cps[q, h][:],
                        V[:, vo : vo + 65],
                        es[:, h * 512 : (h + 1) * 512],
                        start=(t == 0),
                        stop=(t == NT - 1),
                    )

            def new_cps(q):
                cps[q, 0] = psc.tile([65, 512], f32, tag="c0", name="c0")
                cps[q, 1] = psc.tile([65, 512], f32, tag="c1", name="c1")

            recs = {}

            def fin_recip(q, h, final=False):
                # stage 1 of the finish: pull the ones-row denominator
                # out as fp16 and broadcast it across partitions 0:64
                # with a 1-row fp16 matmul.  (The broadcast comes FIRST
                # because custom-DVE ops only work at base partition 0
                # on this hardware, and the builtin reciprocal is ~8
                # cycles/elem — both measured.)  The final block's cast
                # borrows the by-then-idle ScalarE so the tail's DVE
                # chain is shorter.
                den16 = csbpool.tile([65, 512], f16, tag=f"den{h}", name="den")
                with nc.allow_low_precision(
                    reason="softmax denom in fp16: rel err 5e-4 is ample"
                ):
                    if final:
                        nc.scalar.copy(den16[64:65, :], cps[q, h][64:65, :])
                    else:
                        nc.vector.tensor_copy(den16[64:65, :], cps[q, h][64:65, :])
                bc = psq.tile([P, CH], f32, tag="psq", name="bc")
                mm(
                    bc[0:64, :],
                    ones_sb[64:65, :],
                    den16[64:65, :],
                    start=True,
                    stop=True,
                )
                recs[q, h] = bc

            def fin_out(q, h, final=False):
                # stage 2: fast-approx reciprocal of the broadcast
                # denominator (~51 ULP, one DVE pass, 128 lanes), scale
                # the ctx PSUM, and store [64, 512] directly.  The
                # final block stripes its stores over both HWDGE queues.
                bc = recs.pop((q, h))
                rec = rpool.tile([64, 512], f32, tag=f"rec{h}", name="rec")
                nc.vector.reciprocal_approx_fast(rec[:], bc[0:64, :])
                o = opool.tile([64, 512], f32, tag="ot", name="ot")
                nc.vector.tensor_mul(o[:], cps.pop((q, h))[0:64, :], rec[:])
                qs = q * 512
                dst = io["out"][64 * h : 64 * h + 64, qs : qs + 512]
                if final:
                    nc.sync.dma_start(dst[:, 0:256], o[:, 0:256])
                    nc.scalar.dma_start(dst[:, 256:512], o[:, 256:512])
                else:
                    nc.sync.dma_start(dst, o[:])

            # ---- schedule -------------------------------------------
            # S-units (score+exp) drive the cadence; fillers weave
            # between them.  Score batches never exceed the 2-deep sps
            # ring, and every filler lands where its inputs (DMA
            # arrivals, es tiles, psc hand-off) are already resolved.
            warmup(14)
            new_cps(0)
            # xq0 lands first (zero bytes ahead of it on sync), so the
            # Q projection starts immediately; the K chunk-0 pieces
            # follow as xk0 lands on the scalar queue
            proj_q(0)
            proj_k(0, 0, P)
            proj_k(0, P, CH)
            score(0, 0)
            score(0, 1)
            score(0, 2)
            proj_q(1)
            score(0, 3)
            proj_k(1)
            # c-loop: V projections + q0 ctx + remaining projections
            # woven between q0's (and early q1's) exps
            score(0, 4)
            proj_v(0, 0); proj_v(0, 1)
            score(0, 5)
            proj_v(0, 2); proj_v(0, 3)
            score(0, 6)
            ctx(0, 0)
            score(0, 7)
            ctx(0, 1)
            proj_k(2)
            score(1, 0)
            proj_v(1, 0); proj_v(1, 1)
            score(1, 1)
            proj_v(1, 2); proj_v(1, 3)
            score(0, 8)
            proj_k(3)
            score(0, 9)
            ctx(0, 2)
            score(0, 10)
            ctx(0, 3)
            score(0, 11)
            proj_q(2)
            score(1, 2)
            proj_q(3)
            score(1, 3)
            proj_v(2, 0); proj_v(2, 1)
            score(0, 12)
            proj_v(2, 2); proj_v(2, 3)
            score(0, 13)
            ctx(0, 4)
            score(0, 14)
            ctx(0, 5)
            score(0, 15)
            ctx(0, 6)
            score(1, 4)
            ctx(0, 7)
            score(1, 5)
            proj_v(3, 0); proj_v(3, 1)
            score(1, 6)
            proj_v(3, 2); proj_v(3, 3)
            score(1, 7)
            ctx(0, 8); ctx(0, 9)
            score(1, 8)
            ctx(0, 10); ctx(0, 11)
            score(1, 9)
            ctx(0, 12); ctx(0, 13)
            score(1, 10)
            ctx(0, 14); ctx(0, 15)
            score(1, 11)
            fin_recip(0, 0); fin_recip(0, 1)
            score(1, 12)
            fin_out(0, 0)
            score(1, 13)
            fin_out(0, 1)
            new_cps(1)
            score(1, 14)
            ctx(1, 0); ctx(1, 1)
            score(1, 15)
            ctx(1, 2); ctx(1, 3)
            # q2/q3 blocks: ctx consumption lags its exp by ~8 slots so
            # a ctx matmul never waits on a just-finishing exp (that
            # wait was measured at ~370ns/slot); the last block tightens
            # the lag at the end so the post-train tail stays short
            score(2, 0)
            ctx(1, 4); ctx(1, 5)
            score(2, 1)
            ctx(1, 6)
            score(2, 2)
            ctx(1, 7); ctx(1, 8)
            score(2, 3)
            ctx(1, 9)
            score(2, 4)
            ctx(1, 10); ctx(1, 11)
            score(2, 5)
            ctx(1, 12)
            score(2, 6)
            ctx(1, 13); ctx(1, 14)
            score(2, 7)
            ctx(1, 15)
            score(2, 8)
            fin_recip(1, 0); fin_recip(1, 1)
            score(2, 9)
            fin_out(1, 0)
            score(2, 10)
            fin_out(1, 1)
            new_cps(2)
            score(2, 11)
            ctx(2, 0)
            score(2, 12)
            ctx(2, 1)
            score(2, 13)
            ctx(2, 2)
            score(2, 14)
            ctx(2, 3)
            score(2, 15)
            ctx(2, 4)
            score(3, 0)
            ctx(2, 5); ctx(2, 6)
            score(3, 1)
            ctx(2, 7); ctx(2, 8)
            score(3, 2)
            ctx(2, 9); ctx(2, 10)
            score(3, 3)
            ctx(2, 11); ctx(2, 12)
            score(3, 4)
            ctx(2, 13); ctx(2, 14)
            score(3, 5)
            ctx(2, 15)
            fin_recip(2, 0)
            score(3, 6)
            fin_recip(2, 1)
            fin_out(2, 0)
            score(3, 7)
            fin_out(2, 1)
            new_cps(3)
            score(3, 8)
            ctx(3, 0); ctx(3, 1)
            score(3, 9)
            ctx(3, 2); ctx(3, 3)
            score(3, 10)
            ctx(3, 4); ctx(3, 5)
            score(3, 11)
            ctx(3, 6); ctx(3, 7)
            score(3, 12)
            ctx(3, 8); ctx(3, 9)
            score(3, 13)
            ctx(3, 10); ctx(3, 11)
            score(3, 14)
            ctx(3, 12)
            score(3, 15)
            ctx(3, 13)
            ctx(3, 14); ctx(3, 15)
            fin_recip(3, 0, final=True); fin_recip(3, 1, final=True)
            fin_out(3, 0, final=True); fin_out(3, 1, final=True)


def _get_program():
    if "nc" not in _CACHE:
        _CACHE["nc"] = _build_program()
    return _CACHE["nc"]


def _prearrange_xT(x, dtype):
    """[S, DIM] fp32 -> [128, NK*S] with
    [p, c*2048 + k*512 + s] = x[c*512+s, 128k+p]."""
    xT = np.ascontiguousarray(x.T.astype(dtype))  # [512, 2048]
    return np.ascontiguousarray(
        xT.reshape(NK, P, S // CH, CH).transpose(1, 2, 0, 3).reshape(P, NK * S)
    )


def _shard_inputs(query, key, value, Wq, bq, Wk, bk, Wv, bv):
    """Build the 8 per-core input dicts (q/k path fp8, v path fp16)."""
    f8 = ml_dtypes.float8_e3m4
    maps = []
    xP = {}
    for b in range(B):
        xP[b] = (
            _prearrange_xT(query[b], f8),
            _prearrange_xT(key[b], f8),
            _prearrange_xT(value[b], np.float16),
        )

    def wslice(W, g, dtype, scale=1.0):
        # want w[p, 128k + m] = scale * W[128g + m, 128k + p]
        Ws = W[P * g : P * (g + 1), :] * scale  # [m, 512]
        return np.ascontiguousarray(
            Ws.reshape(P, NK, P).transpose(2, 1, 0).reshape(P, DIM).astype(dtype)
        )

    for c in range(N_CORES):
        b, g = c // 4, c % 4
        sl = slice(P * g, P * (g + 1))
        maps.append(
            {
                "xq": xP[b][0],
                "xk": xP[b][1],
                "xv": xP[b][2],
                "wq": wslice(Wq, g, f8, WSCALE),
                "wk": wslice(Wk, g, f8, WSCALE),
                "wv": wslice(Wv, g, np.float16),
                "bqk": np.ascontiguousarray(
                    WSCALE * np.stack([bq[sl], bk[sl]], axis=1), dtype=np.float32
                ),
                "bvb": np.ascontiguousarray(
                    np.broadcast_to(bv[sl], (P, P)), dtype=np.float32
                ),
            }
        )
    return maps


def _numpy_reference(query, key, value, mask, Wq, bq, Wk, bk, Wv, bv):
    """Pure-numpy fallback (only used when the mask isn't all ones)."""
    out = np.empty((B, S, DIM), dtype=np.float32)
    for b in range(B):
        q = (query[b] @ Wq.T + bq).reshape(S, NUM_HEADS, D_HEAD)
        k = (key[b] @ Wk.T + bk).reshape(S, NUM_HEADS, D_HEAD)
        v = (value[b] @ Wv.T + bv).reshape(S, NUM_HEADS, D_HEAD)
        for h in range(NUM_HEADS):
            s = q[:, h, :] @ k[:, h, :].T
            s = np.where(mask[b], s, np.float32(-10000.0))
            s = s / np.float32(np.sqrt(DIM))
            s = s - s.max(axis=-1, keepdims=True)
            e = np.exp(s)
            p = e / e.sum(axis=-1, keepdims=True)
            out[b, :, h * D_HEAD : (h + 1) * D_HEAD] = p @ v[:, h, :]
    return out


LAST_EXEC_NS = None
LAST_RESULTS = None


def kernel(query, key, value, mask, Wq, bq, Wk, bk, Wv, bv):
    global LAST_EXEC_NS, LAST_RESULTS
    query = np.asarray(query, dtype=np.float32)
    key = np.asarray(key, dtype=np.float32)
    value = np.asarray(value, dtype=np.float32)
    mask = np.asarray(mask)
    Wq = np.asarray(Wq, dtype=np.float32)
    bq = np.asarray(bq, dtype=np.float32)
    Wk = np.asarray(Wk, dtype=np.float32)
    bk = np.asarray(bk, dtype=np.float32)
    Wv = np.asarray(Wv, dtype=np.float32)
    bv = np.asarray(bv, dtype=np.float32)

    if not mask.all():
        return _numpy_reference(query, key, value, mask, Wq, bq, Wk, bk, Wv, bv)

    from concourse.bass_utils import run_bass_kernel_spmd

    nc = _get_program()
    in_maps = _shard_inputs(query, key, value, Wq, bq, Wk, bk, Wv, bv)
    trace = os.environ.get("KERNEL_TRACE", "0") == "1"
    tmpdir = os.environ.get("KERNEL_TRACE_DIR") or None
    try:
        res = run_bass_kernel_spmd(
            nc, in_maps, list(range(N_CORES)), trace=trace, tmpdir=tmpdir
        )
    except Exception:
        if not trace:
            raise
        import traceback

        traceback.print_exc()
        res = run_bass_kernel_spmd(nc, in_maps, list(range(N_CORES)), trace=False)
    LAST_EXEC_NS = res.exec_time_ns
    LAST_RESULTS = res
    out = np.empty((B, S, DIM), dtype=np.float32)
    for c in range(N_CORES):
        b, g = c // 4, c % 4
        # device emits [d', q]; untranspose to [q, d']
        o = np.asarray(res.results[c]["out"])
        out[b, :, P * g : P * (g + 1)] = o.T
    return out
